# revision 14
# baseline (speedup 1.0000x reference)
"""CenterLoss kernel for Trainium2 (8 NeuronCores, Bass/Tile).

Problem (fixed shapes):
    features [32768, 512] f32, labels [32768] int64 in [0, 1000), centers [1000, 512] f32
    loss        = 0.5 * sum((features - centers[labels])^2) / B
    new_centers = where(count>0, (1-a)*centers + a*(segsum(features)/count), centers), a=0.5

Strategy
--------
Shard by CLASS RANGE: core j owns classes [125j, 125j+125). Host routes each
batch row to the core owning its label (this is the shard step — a host-side
fancy-index, same as any data-parallel slice). Each core then scatter-adds its
~4096 rows into ONE 128-partition accumulator tile via a one-hot matmul
(onehot[b, c].T @ feat[b, d], accumulated in PSUM across row tiles), so the PE
does 8x less work than a 1000-class one-hot and no cross-core reduction of the
[1000, 512] state is needed -- classes are owned exclusively.

The gather (centers[labels]) is eliminated algebraically:
    sum_i ||f_i - c_{l_i}||^2 = sum||f||^2 - 2*<segsum(f), centers> + sum_c count_c*||c_c||^2
All three terms come from the segment sums/counts we need anyway.

Everything on device is fp32 (matmul operands bitcast to float32r: exact for
the 0/1 one-hot weights; moving operand passes through the PE fp32 path at
1 cycle/row for N>=256). PSUM accumulation is fp32.

Per-core engine budget (T = 36 row-tiles of 128):
    DMA  ~9 MB in (features) -> ~25 us  <- bound (target_regime=memory)
    PE   36 x (512-cycle sums MM + 1-col counts MM) ~ 10-14 us
    DVE  36 x 128-cycle one-hot is_equal + epilogue ~ 8 us
    ACT  36 x 512-cycle square+row-accum (for sum||f||^2) ~ 13 us
"""

import math
import os

import numpy as np

import concourse.bass as bass
import concourse.bacc as bacc
import concourse.mybir as mybir
import concourse.tile as tile
from concourse.bass_utils import run_bass_kernel_spmd

NUM_CLASSES = 1000
FEAT_DIM = 512
ALPHA = 0.5
N_CORES = 8
CPC = NUM_CLASSES // N_CORES  # 125 classes per core
P = 128
G = 4  # 128-row subtiles per DMA group (1 MiB loads)

# matmul operand mode:
#   "f32r" - PE fast fp32 path (1 cyc/row at N>=256), DMA stays 4B/elem
#   "f32"  - exact fp32, 4 cyc/row on PE
#   "bf16" - operands in bf16: halves feature DMA traffic; PSUM accum fp32
MM_DTYPE = os.environ.get("CENTERLOSS_MM_DTYPE", "f32r")

_NC_CACHE: dict = {}


def _mm_dt():
    return {
        "f32r": mybir.dt.float32r,
        "f32": mybir.dt.float32,
        "bf16": mybir.dt.bfloat16,
    }[MM_DTYPE]


def _np_feat_dt():
    import ml_dtypes

    return ml_dtypes.bfloat16 if MM_DTYPE == "bf16" else np.float32


def _build(T: int) -> bass.Bass:
    """One SPMD program; all 8 cores run it on their own shard."""
    f32 = mybir.dt.float32
    mdt = _mm_dt()
    nG = T // G
    nc = bacc.Bacc(None, target_bir_lowering=False)

    feat = nc.declare_dram_parameter("feat", [nG * P, G * FEAT_DIM], mdt, isOutput=False)
    lab2d = nc.declare_dram_parameter("lab2d", [P, T], f32, isOutput=False)
    cent = nc.declare_dram_parameter("cent", [P, FEAT_DIM], f32, isOutput=False)
    out_c = nc.declare_dram_parameter("out_centers", [P, FEAT_DIM], f32, isOutput=True)
    out_p = nc.declare_dram_parameter("out_partials", [P, 4], f32, isOutput=True)

    with tile.TileContext(nc) as tc:
        with (
            tc.tile_pool(name="singles", bufs=1) as singles,
            tc.tile_pool(name="ftiles", bufs=3) as ftiles,
            tc.tile_pool(name="ohpool", bufs=4) as ohpool,
            tc.tile_pool(name="sqpool", bufs=2) as sqpool,
            tc.tile_pool(name="psum", bufs=1, space="PSUM") as psum,
        ):
            # constants / whole-kernel inputs
            iota_i = singles.tile([P, P], mybir.dt.int32)
            nc.gpsimd.iota(iota_i[:], [[1, P]], channel_multiplier=0)
            iota_f = singles.tile([P, P], f32)
            nc.vector.tensor_copy(iota_f[:], iota_i[:])

            lab_s = singles.tile([P, T], f32)
            nc.sync.dma_start(out=lab_s[:], in_=lab2d[:])
            cent_s = singles.tile([P, FEAT_DIM], f32)
            nc.sync.dma_start(out=cent_s[:], in_=cent[:])
            # N=2 keeps the counts matmul legal under fp32r (even free count)
            ones_f = singles.tile([P, 2], f32)
            nc.vector.memset(ones_f[:], 1.0)
            ones_s = singles.tile([P, 2], mdt)
            nc.vector.tensor_copy(ones_s[:], ones_f[:])
            ssq_cols = singles.tile([P, T], f32)

            sums_ps = psum.tile([P, FEAT_DIM], f32)   # one full PSUM bank
            counts_ps = psum.tile([P, 2], f32)        # its own bank (padded)

            feat_v = feat[:].rearrange("(g p) m -> g p m", p=P)
            for g in range(nG):
                ft = ftiles.tile([P, G, FEAT_DIM], mdt)
                nc.sync.dma_start(out=ft[:], in_=feat_v[g])
                for u in range(G):
                    t = g * G + u
                    oh = ohpool.tile([P, P], mdt)
                    nc.vector.tensor_scalar(
                        oh[:], iota_f[:], lab_s[:, t : t + 1], None,
                        mybir.AluOpType.is_equal,
                    )
                    nc.tensor.matmul(
                        sums_ps[:], lhsT=oh[:], rhs=ft[:, u, :],
                        start=(t == 0), stop=(t == T - 1),
                    )
                    nc.tensor.matmul(
                        counts_ps[:], lhsT=oh[:], rhs=ones_s[:],
                        start=(t == 0), stop=(t == T - 1),
                    )
                    sq = sqpool.tile([P, FEAT_DIM], f32)
                    nc.scalar.activation(
                        sq[:], ft[:, u, :], mybir.ActivationFunctionType.Square,
                        accum_out=ssq_cols[:, t : t + 1],
                    )

            # ---- epilogue: center blend + loss partials ----
            counts_f = singles.tile([P, 1], f32)
            nc.vector.tensor_copy(counts_f[:], counts_ps[:, 0:1])
            safe = singles.tile([P, 1], f32)
            nc.vector.tensor_scalar_max(safe[:], counts_f[:], 1.0)
            recip = singles.tile([P, 1], f32)
            nc.vector.reciprocal(recip[:], safe[:])
            mean = singles.tile([P, FEAT_DIM], f32)
            nc.vector.tensor_scalar_mul(mean[:], sums_ps[:], recip[:, :1])
            mask = singles.tile([P, 1], f32)
            nc.vector.tensor_scalar(
                mask[:], counts_f[:], 0.0, None, mybir.AluOpType.is_gt
            )
            diff = singles.tile([P, FEAT_DIM], f32)
            nc.vector.tensor_tensor(
                out=diff[:], in0=mean[:], in1=cent_s[:], op=mybir.AluOpType.subtract
            )
            dm = singles.tile([P, FEAT_DIM], f32)
            # dm = (diff * mask) * ALPHA
            nc.vector.tensor_scalar(
                dm[:], diff[:], mask[:, :1], ALPHA,
                mybir.AluOpType.mult, mybir.AluOpType.mult,
            )
            newc = singles.tile([P, FEAT_DIM], f32)
            nc.vector.tensor_tensor(
                out=newc[:], in0=cent_s[:], in1=dm[:], op=mybir.AluOpType.add
            )
            nc.sync.dma_start(out=out_c[:], in_=newc[:])

            partials = singles.tile([P, 4], f32)
            nc.vector.tensor_reduce(
                partials[:, 0:1], ssq_cols[:], axis=mybir.AxisListType.X,
                op=mybir.AluOpType.add,
            )
            # (tensor_tensor_reduce crashes the exec unit on this HW path;
            # use separate mul + reduce)
            scr = singles.tile([P, FEAT_DIM], f32)
            nc.vector.tensor_tensor(
                out=scr[:], in0=sums_ps[:], in1=cent_s[:], op=mybir.AluOpType.mult
            )
            nc.vector.tensor_reduce(
                partials[:, 1:2], scr[:], axis=mybir.AxisListType.X,
                op=mybir.AluOpType.add,
            )
            scr2 = singles.tile([P, FEAT_DIM], f32)
            csq = singles.tile([P, 1], f32)
            nc.scalar.activation(
                scr2[:], cent_s[:], mybir.ActivationFunctionType.Square,
                accum_out=csq[:],
            )
            nc.vector.tensor_tensor(
                out=partials[:, 2:3], in0=csq[:], in1=counts_f[:],
                op=mybir.AluOpType.mult,
            )
            nc.vector.memset(partials[:, 3:4], 0.0)
            nc.sync.dma_start(out=out_p[:], in_=partials[:])

    nc.compile()
    return nc


def _get_nc(T: int) -> bass.Bass:
    if T not in _NC_CACHE:
        _NC_CACHE[T] = _build(T)
    return _NC_CACHE[T]


def _shard_inputs(features, labels, centers):
    """Route rows to the core owning their class; build per-core maps."""
    order = np.argsort(labels, kind="stable")
    sl = labels[order]
    bounds = np.searchsorted(sl, np.arange(0, NUM_CLASSES + 1, CPC))
    n_max = int(np.diff(bounds).max())
    T = max(G, math.ceil(n_max / P))
    T = math.ceil(T / G) * G
    Bp = T * P
    nG = T // G

    fdt = _np_feat_dt()
    in_maps = []
    for j in range(N_CORES):
        idx = order[bounds[j] : bounds[j + 1]]
        n = len(idx)
        fj = np.zeros((Bp, FEAT_DIM), fdt)
        fj[:n] = features[idx].astype(fdt)
        lj = np.full(Bp, P - 1, np.float32)  # pad rows -> dummy class 127
        lj[:n] = (sl[bounds[j] : bounds[j + 1]] - j * CPC).astype(np.float32)
        # device layout: row g*G*128 + u*128 + p -> feat[g*128+p, u*512:(u+1)*512]
        fdev = np.ascontiguousarray(
            fj.reshape(nG, G, P, FEAT_DIM).transpose(0, 2, 1, 3)
        ).reshape(nG * P, G * FEAT_DIM)
        ldev = np.ascontiguousarray(lj.reshape(T, P).T)
        cj = np.zeros((P, FEAT_DIM), np.float32)
        cj[:CPC] = centers[j * CPC : (j + 1) * CPC]
        in_maps.append({"feat": fdev, "lab2d": ldev, "cent": cj})
    return in_maps, T


def _run(features, labels, centers, trace=False):
    in_maps, T = _shard_inputs(features, labels, centers)
    nc = _get_nc(T)
    out = run_bass_kernel_spmd(
        nc, in_maps, core_ids=list(range(N_CORES)), trace=trace
    )
    res = out.results
    new_centers = np.empty((NUM_CLASSES, FEAT_DIM), np.float32)
    ssq = dot = cn = 0.0
    for j in range(N_CORES):
        new_centers[j * CPC : (j + 1) * CPC] = res[j]["out_centers"][:CPC]
        pr = np.asarray(res[j]["out_partials"], dtype=np.float64)
        ssq += pr[:, 0].sum()
        dot += pr[:, 1].sum()
        cn += pr[:, 2].sum()
    B = features.shape[0]
    loss = np.float32(0.5 * (ssq - 2.0 * dot + cn) / B)
    return (loss, new_centers), out


def kernel(features, labels, centers):
    features = np.ascontiguousarray(np.asarray(features), dtype=np.float32)
    labels = np.asarray(labels).astype(np.int64)
    centers = np.ascontiguousarray(np.asarray(centers), dtype=np.float32)
    (loss, new_centers), _ = _run(features, labels, centers, trace=False)
    return loss, new_centers
